# revision 5
# baseline (speedup 1.0000x reference)
"""CFM contrastive loss on 8 TRN2 NeuronCores.

loss = -mean(diag(log_softmax(logits))),  logits[i,j] = 2*z1_i.z2_j - |z1_i|^2 - |z2_j|^2

The |z1_i|^2 term cancels between the logsumexp and the diagonal, so with
t[i,j] = 2*z1_i.z2_j - |z2_j|^2 the loss is mean_i(log(sum_j exp(t_ij)) - t_ii).

Sharding: z1 rows are split across 8 cores (1024 rows each); every core reads
all of z2.  Per core, rowsum_i = sum_j exp(t_ij) is computed as

    sum_j exp(g_ij - C) * w_j,   g = 2*z1 @ z2^T,  w_j = exp(C - |z2_j|^2)

with C = 100 keeping both factors inside bf16/fp32 range (max g = 176, so
exp(g-C) <= e^76; terms that underflow are < 1e-9 of any row's sum).  This
splits the work cleanly across three engines with no PE prefill pass:
  - PE: one K=128 bf16 matmul per PSUM bank (g into PSUM fp32),
  - ScalarE: exp(psum - C) straight out of PSUM into a bf16 SBUF tile
    (bias is the per-instruction constant -C; no accum_out stall),
  - VectorE: tensor_tensor_reduce multiplies by w (replicated across
    partitions) and row-sum-accumulates in fp32, in 4x DVE perf mode.
The host pre-transposes/casts the operands (layout prep only), and finishes
with log + mean in float64, plus the cheap O(N*D) diagonal term.
"""

import numpy as np
import ml_dtypes

N, D = 8192, 128
NCORES = 8
SHARD = N // NCORES      # 1024 z1 rows per core
ITILES = SHARD // 128    # 8 i-tiles per core
JCHUNK = 2048            # PSUM chunk = 4 banks of 512 fp32
NCHUNKS = N // JCHUNK    # 4 chunks of j per i-tile
CSHIFT = 100.0           # range shift: exp(g - C) * exp(C - sq2)
BF16 = ml_dtypes.bfloat16

_NC_CACHE = None


def _build_nc():
    import concourse.mybir as mybir
    import concourse.tile as tile
    from concourse import bacc

    nc = bacc.Bacc(None, target_bir_lowering=False)

    z1t2 = nc.dram_tensor("z1t2", [128, SHARD], mybir.dt.bfloat16, kind="ExternalInput")
    z2t = nc.dram_tensor("z2t", [128, N], mybir.dt.bfloat16, kind="ExternalInput")
    wrep = nc.dram_tensor("wrep", [128, N], mybir.dt.bfloat16, kind="ExternalInput")
    rs = nc.dram_tensor(
        "rs", [128, ITILES * NCHUNKS], mybir.dt.float32, kind="ExternalOutput"
    )

    EXP = mybir.ActivationFunctionType.Exp

    with tile.TileContext(nc) as tc:
        with (
            tc.tile_pool(name="const", bufs=1) as cpool,
            tc.tile_pool(name="esc", bufs=2) as epool,
            tc.tile_pool(name="scr", bufs=2) as spool,
            tc.tile_pool(name="psum", bufs=2, space="PSUM") as ppool,
        ):
            z1t2_sb = cpool.tile([128, SHARD], mybir.dt.bfloat16)
            z2t_sb = cpool.tile([128, N], mybir.dt.bfloat16)
            w_sb = cpool.tile([128, N], mybir.dt.bfloat16)
            rs_parts = cpool.tile([128, ITILES * NCHUNKS], mybir.dt.float32)
            bias_sb = cpool.tile([128, 1], mybir.dt.float32)

            nc.gpsimd.memset(bias_sb[:], -CSHIFT)

            # interleave so chunk c's z2/w land just ahead of its compute
            nc.sync.dma_start(z1t2_sb[:], z1t2[:])
            for q in range(NCHUNKS):
                sl = slice(q * JCHUNK, (q + 1) * JCHUNK)
                nc.sync.dma_start(z2t_sb[:, sl], z2t[:, sl])
                nc.sync.dma_start(w_sb[:, sl], wrep[:, sl])

            for it in range(ITILES):
                lhsT = z1t2_sb[:, it * 128 : (it + 1) * 128]
                for c in range(NCHUNKS):
                    ps = ppool.tile([128, JCHUNK], mybir.dt.float32)
                    for b in range(4):
                        j0 = c * JCHUNK + b * 512
                        nc.tensor.matmul(
                            ps[:, b * 512 : (b + 1) * 512],
                            lhsT,
                            z2t_sb[:, j0 : j0 + 512],
                            start=True,
                            stop=True,
                        )
                    e_tile = epool.tile([128, JCHUNK], mybir.dt.bfloat16)
                    nc.scalar.activation(e_tile[:], ps[:], EXP, bias=bias_sb[:])
                    col = it * NCHUNKS + c
                    scr = spool.tile([128, JCHUNK], mybir.dt.bfloat16)
                    nc.vector.scalar_tensor_tensor(
                        out=scr[:],
                        in0=e_tile[:],
                        scalar=1.0,
                        in1=w_sb[:, c * JCHUNK : (c + 1) * JCHUNK],
                        op0=mybir.AluOpType.mult,
                        op1=mybir.AluOpType.mult,
                        accum_out=rs_parts[:, col : col + 1],
                    )

            # chunk partials go out raw; the host sums the NCHUNKS columns
            nc.sync.dma_start(rs[:], rs_parts[:])

    nc.compile()
    return nc


def _get_nc():
    global _NC_CACHE
    if _NC_CACHE is None:
        _NC_CACHE = _build_nc()
    return _NC_CACHE


def _prep_inputs(z1, z2):
    z1 = np.asarray(z1, dtype=np.float32)
    z2 = np.asarray(z2, dtype=np.float32)
    z2b = z2.astype(BF16)
    z2t = np.ascontiguousarray(z2b.T)  # [128, N] bf16
    sq2 = (z2b.astype(np.float64) ** 2).sum(axis=-1)  # from the bf16 values
    w = np.exp(CSHIFT - sq2).astype(np.float32).astype(BF16)
    wrep = np.ascontiguousarray(np.broadcast_to(w[None, :], (128, N)))
    in_maps = []
    for c in range(NCORES):
        z1s = z1[c * SHARD : (c + 1) * SHARD]
        z1t2 = np.ascontiguousarray((2.0 * z1s.astype(np.float64)).astype(BF16).T)
        in_maps.append({"z1t2": z1t2, "z2t": z2t, "wrep": wrep})
    return in_maps


def _finish(z1, z2, rs_list):
    # rs[p, it*NCHUNKS+c] = chunk-c partial rowsum of shard row it*128+p
    rows = np.concatenate(
        [
            np.asarray(r["rs"], np.float64)
            .reshape(128, ITILES, NCHUNKS)
            .sum(axis=2)
            .T.reshape(-1)
            for r in rs_list
        ]
    )
    z1 = np.asarray(z1, dtype=np.float64)
    z2 = np.asarray(z2, dtype=np.float64)
    sq2 = (z2.astype(BF16).astype(np.float64) ** 2).sum(axis=-1)
    tdiag = 2.0 * (z1 * z2).sum(axis=-1) - sq2
    loss = np.mean(np.log(rows) - tdiag)
    return np.asarray(loss, dtype=np.float32)


def _ensure_hook_shim():
    """bass_utils imports antenv.axon_hooks whenever tracing is requested
    (e.g. via a BASS_TRACE env var); this image's antenv lacks that module.
    Provide an inert registry so tracing degrades to a warning instead of an
    ImportError.  A previously installed real shim is left untouched."""
    import sys

    try:
        import antenv.axon_hooks  # noqa: F401
    except ImportError:
        import types

        import antenv

        mod = types.ModuleType("antenv.axon_hooks")
        mod._hook = None
        mod.set_axon_ntff_profile_hook = lambda h: setattr(mod, "_hook", h)
        mod.get_axon_ntff_profile_hook = lambda: mod._hook
        sys.modules["antenv.axon_hooks"] = mod
        antenv.axon_hooks = mod


def _run(z1, z2, **spmd_kwargs):
    _ensure_hook_shim()
    from concourse.bass_utils import run_bass_kernel_spmd

    in_maps = _prep_inputs(z1, z2)
    res = run_bass_kernel_spmd(
        _get_nc(), in_maps, core_ids=list(range(NCORES)), **spmd_kwargs
    )
    return _finish(z1, z2, res.results), res


def kernel(z1, z2):
    loss, _ = _run(z1, z2)
    return loss
